# revision 11
# baseline (speedup 1.0000x reference)
# kernel.py — nn_GAT: 2-layer multi-head GAT on 8 TRN2 NeuronCores.
#
# Structure exploited: the reference edge lists are "shift graphs":
#   src = repeat(arange(N), 16); dst = (src + offs[k]) % N, offs[k] = base + 37*k.
# This turns every gather/scatter into fixed circular shifts, and (since every
# node has exactly DEG=16 out-edges, contiguous in the edge list) the segment
# softmax into a dense row softmax over k. The kernel derives the offsets from
# the actual edge tensors at call time and falls back to a numpy reference
# implementation if the structure does not hold.
#
# Sharding: nodes are partitioned into 8 contiguous ranges of 6250 rows. Each
# core receives its row range plus a wrap-around halo (offsets < 868 per layer,
# two layers) and computes everything locally — no collectives.
import json
import math
import numpy as np

N = 50000
DEG = 16
IN_DIM = 256
HID = 256
OUT_DIM = 64
HEADS = 4
ET = 2
HD = 64
ALPHA = 0.2
NCORES = 8
ROWS = N // NCORES          # 6250
L0 = 8192                   # X_ext rows per core (>= 6250 + 2*867 + slack)
L1 = 7168                   # layer-0 output extent (>= 6250 + 867), 14*512
LOUT = 6656                 # layer-1 agg extent (13*512 >= 6250)
HT1_PAD = 7680              # layer-1 hT read extent (6655 + 867 = 7522 max)
ECLAMP = 80.0               # safety clamp on attention logits (exp w/o max-sub)


# ----------------------------------------------------------------------------
# numpy fallback (exact reference semantics) — used if edges are unstructured.
# ----------------------------------------------------------------------------
def _np_sparse_gat(X, src, dst, W, a):
    h = X @ W
    D = h.shape[1]
    s_src = h @ a[:D]
    s_dst = h @ a[D:]
    e = s_src[src] + s_dst[dst]
    e = np.where(e >= 0, e, ALPHA * e)
    E = e.reshape(N, DEG)  # requires src == repeat(arange(N), DEG)
    m = E.max(axis=1, keepdims=True)
    ex = np.exp(E - m)
    att = ex / ex.sum(axis=1, keepdims=True)
    hd = h[dst].reshape(N, DEG, D)
    return np.einsum('nk,nkd->nd', att, hd)


def _np_sparse_gat_general(X, src, dst, W, a):
    h = X @ W
    D = h.shape[1]
    s = h @ a[:D]
    t = h @ a[D:]
    e = s[src] + t[dst]
    e = np.where(e >= 0, e, ALPHA * e)
    m = np.full(N, -np.inf, np.float32)
    np.maximum.at(m, src, e)
    ex = np.exp(e - m[src])
    den = np.zeros(N, np.float32)
    np.add.at(den, src, ex)
    att = ex / den[src]
    out = np.zeros((N, D), np.float32)
    np.add.at(out, src, att[:, None] * h[dst])
    return out


def _elu(x):
    return np.where(x > 0, x, np.expm1(np.minimum(x, 0.0)))


def _np_forward(X, edges0, edges1, W0, a0, W1, a1, etw0, etb0, etw1, etb1,
                gat=_np_sparse_gat_general):
    graphs = (edges0, edges1)
    cols = []
    for j in range(ET):
        src, dst = graphs[j][0], graphs[j][1]
        heads = [_elu(gat(X, src, dst, W0[j, k], a0[j, k])) for k in range(HEADS)]
        cols.append(np.concatenate(heads, axis=1))
    X1 = _elu(np.stack(cols, axis=-1) @ etw0 + etb0)
    cols = []
    for j in range(ET):
        src, dst = graphs[j][0], graphs[j][1]
        acc = gat(X1, src, dst, W1[j, 0], a1[j, 0])
        for k in range(1, HEADS):
            acc = acc + gat(X1, src, dst, W1[j, k], a1[j, k])
        cols.append(_elu(acc / HEADS))
    return _elu(np.stack(cols, axis=-1) @ etw1 + etb1).astype(np.float32)


def _derive_offsets(edges):
    """Return offs[16] if edges match the shift-graph structure, else None."""
    src, dst = np.asarray(edges[0]), np.asarray(edges[1])
    E = src.shape[0]
    if E != N * DEG:
        return None
    if not np.array_equal(src[:2 * DEG],
                          np.repeat(np.arange(2, dtype=src.dtype), DEG)):
        return None
    offs = ((dst[:DEG].astype(np.int64) - src[:DEG].astype(np.int64)) % N)
    step = offs[1] - offs[0]
    if step <= 0 or not np.array_equal(np.diff(offs), np.full(DEG - 1, step)):
        return None
    if offs[0] <= 0 or offs[-1] >= 868:
        return None
    # full verification (vectorized, cheap)
    exp_src = np.repeat(np.arange(N, dtype=np.int64), DEG)
    exp_dst = (exp_src + np.tile(offs, N)) % N
    if not np.array_equal(src.astype(np.int64), exp_src):
        return None
    if not np.array_equal(dst.astype(np.int64), exp_dst):
        return None
    return offs.astype(int)


# ----------------------------------------------------------------------------
# Bass kernel construction
# ----------------------------------------------------------------------------
_CACHE = {}


def _patch_waitsplit(bass):
    """This container's walrus only accepts one sem wait per instruction;
    split multi-wait instructions into single-wait NoOp chains."""
    if getattr(bass.Bass, '_waitsplit_patched', False):
        return
    _orig = bass.Bass.to_json_bytes

    def patched(self):
        d = json.loads(_orig(self))
        nid = [0]
        for f in d['functions']:
            for bb in f['blocks']:
                out = []
                for inst in bb['instructions']:
                    si = inst.get('sync_info') or {}
                    waits = si.get('on_wait') or []
                    if len(waits) > 1:
                        keep = waits[-1]
                        for w in waits[:-1]:
                            nid[0] += 1
                            out.append({
                                'debug': inst.get('debug', 0),
                                'engine': inst['engine'],
                                'ins': [], 'outs': [],
                                'name': f"I-ws-{nid[0]}", 'opcode': 'NoOp',
                                'sync_info': {'on_update': [], 'on_wait': [w]}})
                        si['on_wait'] = [keep]
                    out.append(inst)
                bb['instructions'] = out
        return json.dumps(d).encode()

    bass.Bass.to_json_bytes = patched
    bass.Bass._waitsplit_patched = True


def _build_bass(offs0, offs1, etw0, etb0, etw1, etb1):
    import concourse.bass as bass
    import concourse.mybir as mybir
    import concourse.tile as tile
    from concourse import masks
    from concourse.bass_types import AP

    _patch_waitsplit(bass)
    offs0 = [int(o) for o in offs0]
    offs1 = [int(o) for o in offs1]

    F32 = mybir.dt.float32
    OP = mybir.AluOpType
    AF = mybir.ActivationFunctionType

    nc = bass.Bass('TRN2')

    # ---- DRAM I/O ----
    Xe = nc.dram_tensor('Xe', [L0, IN_DIM], F32, kind='ExternalInput')
    w0 = [[nc.dram_tensor(f'w0_{j}_{p}', [IN_DIM, 128], F32, kind='ExternalInput')
           for p in range(2)] for j in range(2)]
    w1 = [[nc.dram_tensor(f'w1_{j}_{p}', [HID, 128], F32, kind='ExternalInput')
           for p in range(2)] for j in range(2)]
    wa0 = nc.dram_tensor('wa0', [IN_DIM, 16], F32, kind='ExternalInput')
    wa1 = nc.dram_tensor('wa1', [HID, 16], F32, kind='ExternalInput')
    selK = nc.dram_tensor('selK', [64, DEG * 128], F32, kind='ExternalInput')
    fold16 = nc.dram_tensor('fold16', [64, 2], F32, kind='ExternalInput')
    fold128 = nc.dram_tensor('fold128', [128, 64], F32, kind='ExternalInput')
    outD = nc.dram_tensor('outT', [64, LOUT], F32, kind='ExternalOutput')
    # DRAM scratch (per-core private)
    sD = nc.dram_tensor('sD', [16, L0], F32)     # s^T bounce buffer
    dD = nc.dram_tensor('dD', [2, L0], F32)      # 1/den bounce buffer

    offs = (offs0, offs1)
    NCH0 = L1 // 512        # 14 chunks of agg output, layer 0
    NCH1 = LOUT // 512      # 13 chunks, layer 1

    with tile.TileContext(nc) as tc:
        with tc.tile_pool(name='persist', bufs=1) as pp, \
             tc.tile_pool(name='work', bufs=2) as wp, \
             tc.tile_pool(name='psh', bufs=4, space='PSUM') as psh, \
             tc.tile_pool(name='psx', bufs=2, space='PSUM') as psx, \
             tc.tile_pool(name='psacc', bufs=2, space='PSUM') as psa:

            ident = pp.tile([128, 128], F32, tag='ident')
            masks.make_identity(nc, ident[:])
            sel_sb = pp.tile([64, DEG * 128], F32, tag='sel')
            nc.sync.dma_start(sel_sb[:], selK.ap())
            f16_sb = pp.tile([64, 2], F32, tag='f16')
            nc.sync.dma_start(f16_sb[:], fold16.ap())
            f128_sb = pp.tile([128, 64], F32, tag='f128')
            nc.sync.dma_start(f128_sb[:], fold128.ap())

            hT2 = pp.tile([128, L0], F32, tag='hT2')
            attT = pp.tile([64, L1], F32, tag='attT')
            ssbc = pp.tile([64, 512], F32, tag='ssbc')
            dbc = pp.tile([64, 512], F32, tag='dbc')
            tmp = pp.tile([128, 512], F32, tag='tmp')
            tmp2 = pp.tile([128, 512], F32, tag='tmp2')
            ost = pp.tile([64, 512], F32, tag='ost')
            st16 = pp.tile([16, 512], F32, tag='st16')
            nc.vector.memset(attT[:], 0.0)
            nc.vector.memset(ssbc[:], 0.0)
            nc.vector.memset(dbc[:], 0.0)
            X1T = [pp.tile([128, L1], F32, tag=f'X1T{kt}', name=f'X1T{kt}') for kt in range(2)]
            wsb = [pp.tile([128, 128], F32, tag=f'wsb{kt}', name=f'wsb{kt}') for kt in range(2)]
            wasb = [pp.tile([128, 16], F32, tag=f'wasb{kt}', name=f'wasb{kt}') for kt in range(2)]

            def copy_engine(i):
                return nc.vector if (i % 2 == 0) else nc.scalar

            def copyout(i, dst_ap, src_ap):
                if i % 2 == 0:
                    nc.vector.tensor_copy(dst_ap, src_ap)
                else:
                    nc.scalar.copy(dst_ap, src_ap)

            # ---------- scores s^T for a layer ----------
            def build_scores(xt_tiles, wa_dram, extent):
                for kt in range(2):
                    nc.sync.dma_start(wasb[kt][:], wa_dram.ap()[128 * kt:128 * (kt + 1), :])
                nch = extent // 512
                for ch in range(nch):
                    ps = psh.tile([128, 512], F32, tag='ps', name='ps_s')[0:16, :]
                    for kt in range(2):
                        nc.tensor.matmul(ps[:], wasb[kt][:],
                                         xt_tiles[kt][:, 512 * ch:512 * (ch + 1)],
                                         start=(kt == 0), stop=(kt == 1))
                    copyout(ch, st16[:], ps[:])
                    nc.sync.dma_start(sD.ap()[:, 512 * ch:512 * (ch + 1)], st16[:])

            # ---------- hT2 for one (j, pair) ----------
            def build_h(xt_tiles, w_dram, extent):
                for kt in range(2):
                    nc.sync.dma_start(wsb[kt][:], w_dram.ap()[128 * kt:128 * (kt + 1), :])
                nch = extent // 512
                for sc in range(0, nch, 4):
                    hi = min(sc + 4, nch)
                    pts = [psh.tile([128, 512], F32, tag='ps', name=f'psh{i}') for i in range(hi - sc)]
                    for kt in range(2):
                        for i, ch in enumerate(range(sc, hi)):
                            nc.tensor.matmul(pts[i][:], wsb[kt][:],
                                             xt_tiles[kt][:, 512 * ch:512 * (ch + 1)],
                                             start=(kt == 0), stop=(kt == 1))
                    for i, ch in enumerate(range(sc, hi)):
                        copyout(ch, hT2[:, 512 * ch:512 * (ch + 1)], pts[i][:])

            # ---------- attention att^T for one (j, pair) ----------
            def build_att(j, pair, ext_att):
                # gather shifted s_dst rows and broadcast s_src rows from DRAM
                base = offs[j][0]
                kstep = offs[j][1] - offs[j][0]
                for hh in range(2):
                    head = 2 * pair + hh
                    rsrc = 2 * (4 * j + head)
                    rdst = rsrc + 1
                    hb = 32 * hh
                    # G: shifted s_dst gather -> attT rows of this head
                    src_g = AP(sD.ap().tensor, sD.ap().offset + rdst * L0 + base,
                               [[kstep, DEG], [1, ext_att]])
                    nc.sync.dma_start(attT[hb:hb + 16, 0:ext_att], src_g)
                for ch in range(ext_att // 512):
                    for hh in range(2):
                        head = 2 * pair + hh
                        rsrc = 2 * (4 * j + head)
                        src_b = AP(sD.ap().tensor,
                                   sD.ap().offset + rsrc * L0 + 512 * ch,
                                   [[0, DEG], [1, 512]])
                        nc.sync.dma_start(ssbc[32 * hh:32 * hh + 16, :], src_b)
                    sl = slice(512 * ch, 512 * (ch + 1))
                    nc.vector.scalar_tensor_tensor(
                        out=attT[:, sl], in0=attT[:, sl], scalar=1.0, in1=ssbc[:],
                        op0=OP.mult, op1=OP.add)
                # lrelu slope 0.2 (HW Lrelu table ignores alpha): max(0.2*x, x)
                nc.vector.scalar_tensor_tensor(
                    out=attT[:, 0:ext_att], in0=attT[:, 0:ext_att], scalar=ALPHA,
                    in1=attT[:, 0:ext_att], op0=OP.mult, op1=OP.max)
                nc.vector.tensor_scalar(out=attT[:, 0:ext_att], in0=attT[:, 0:ext_att],
                                        scalar1=ECLAMP, scalar2=None, op0=OP.min)
                nc.scalar.activation(attT[:, 0:ext_att], attT[:, 0:ext_att], AF.Exp)
                # denominators: fold16^T @ ex -> [2, ext]
                nch = ext_att // 512
                for ch in range(nch):
                    ps = psh.tile([128, 512], F32, tag='ps', name='ps_d')[0:2, :]
                    nc.tensor.matmul(ps[:], f16_sb[:],
                                     attT[:, 512 * ch:512 * (ch + 1)],
                                     start=True, stop=True)
                    nc.vector.reciprocal(st16[0:2, :], ps[:])
                    nc.sync.dma_start(dD.ap()[:, 512 * ch:512 * (ch + 1)], st16[0:2, :])
                for ch in range(nch):
                    src_d = AP(dD.ap().tensor, dD.ap().offset + 512 * ch,
                               [[L0, 2], [0, 2 * DEG], [1, 512]])
                    nc.sync.dma_start(dbc[:], src_d)
                    nc.vector.tensor_mul(attT[:, 512 * ch:512 * (ch + 1)],
                                         attT[:, 512 * ch:512 * (ch + 1)], dbc[:])

            # ---------- aggregation for one (j, pair) chunk -> psum acc ----------
            def agg_chunk(j, ch):
                pacc = psa.tile([128, 512], F32, tag='pacc')
                for k in range(DEG):
                    o = offs[j][k]
                    psb_t = psx.tile([128, 512], F32, tag='ps_b')
                    nc.tensor.matmul(psb_t[:], sel_sb[:, 128 * k:128 * (k + 1)],
                                     attT[:, 512 * ch:512 * (ch + 1)],
                                     start=True, stop=True)
                    hsl = hT2[:, 512 * ch + o:512 * ch + o + 512]
                    if k == 0:
                        nc.vector.tensor_mul(pacc[:], hsl, psb_t[:])
                    else:
                        nc.vector.tensor_mul(tmp[:], hsl, psb_t[:])
                        nc.vector.tensor_add(pacc[:], pacc[:], tmp[:])
                return pacc

            # ---------- elu: out = elu(in) (chunk [P,512]); uses tmp2 ----------
            def elu_chunk(dst_ap, src_ap, P):
                t2 = tmp2[0:P, :]
                nc.vector.tensor_scalar(out=t2, in0=src_ap, scalar1=0.0,
                                        scalar2=None, op0=OP.min)
                nc.scalar.activation(t2, t2, AF.Exp)
                nc.vector.tensor_scalar(out=dst_ap, in0=src_ap, scalar1=0.0,
                                        scalar2=None, op0=OP.max)
                nc.vector.scalar_tensor_tensor(out=dst_ap, in0=t2, scalar=-1.0,
                                               in1=dst_ap, op0=OP.add, op1=OP.add)
                return dst_ap

            # ================= LAYER 0 =================
            with tc.tile_pool(name='l0', bufs=1) as l0p:
                XeT = [l0p.tile([128, L0], F32, tag=f'XeT{kt}', name=f'XeT{kt}') for kt in range(2)]
                # transpose X into XeT
                for b in range(L0 // 128):
                    xr = wp.tile([128, IN_DIM], F32, tag='xrow')
                    nc.sync.dma_start(xr[:], Xe.ap()[128 * b:128 * (b + 1), :])
                    for kt in range(2):
                        pt = psh.tile([128, 512], F32, tag='ps', name='ps_t')[:, 0:128]
                        nc.tensor.transpose(pt[:], xr[:, 128 * kt:128 * (kt + 1)],
                                            ident[:])
                        copyout(b + kt, XeT[kt][:, 128 * b:128 * (b + 1)], pt[:])
                build_scores(XeT, wa0, L0)
                for pair in range(2):
                    for j in range(2):
                        build_h(XeT, w0[j][pair], L0)
                        build_att(j, pair, L1)
                        for ch in range(NCH0):
                            pacc = agg_chunk(j, ch)
                            sl = slice(512 * ch, 512 * (ch + 1))
                            e = elu_chunk(tmp[:], pacc[:], 128)
                            if j == 0:
                                nc.vector.tensor_scalar(
                                    out=X1T[pair][:, sl], in0=e, scalar1=float(etw0[0]),
                                    scalar2=float(etb0), op0=OP.mult, op1=OP.add)
                            else:
                                nc.vector.scalar_tensor_tensor(
                                    out=X1T[pair][:, sl], in0=e, scalar=float(etw0[1]),
                                    in1=X1T[pair][:, sl], op0=OP.mult, op1=OP.add)
                    # after both j: final elu on X1T[pair]
                    for ch in range(NCH0):
                        sl = slice(512 * ch, 512 * (ch + 1))
                        elu_chunk(X1T[pair][:, sl], X1T[pair][:, sl], 128)

            # ================= LAYER 1 =================
            with tc.tile_pool(name='l1', bufs=1) as l1p:
                Fst = [l1p.tile([64, LOUT], F32, tag=f'Fst{j}', name=f'Fst{j}') for j in range(2)]
                build_scores(X1T, wa1, L1)
                for j in range(2):
                    for pair in range(2):
                        build_h(X1T, w1[j][pair], L1)
                        # zero pad region of hT2 [L1, HT1_PAD)
                        nc.vector.memset(hT2[:, L1:HT1_PAD], 0.0)
                        build_att(j, pair, L1)
                        for ch in range(NCH1):
                            pacc = agg_chunk(j, ch)
                            sl = slice(512 * ch, 512 * (ch + 1))
                            # fold heads (x 1/4) : [64, 512]
                            nc.vector.tensor_copy(tmp[:], pacc[:])
                            pf = psh.tile([128, 512], F32, tag='ps', name='ps_f')[0:64, :]
                            nc.tensor.matmul(pf[:], f128_sb[:], tmp[:],
                                             start=True, stop=True)
                            if pair == 0:
                                nc.vector.tensor_copy(Fst[j][:, sl], pf[:])
                            else:
                                nc.vector.tensor_add(Fst[j][:, sl], Fst[j][:, sl], pf[:])
                # combine over j with elu
                for ch in range(NCH1):
                    sl = slice(512 * ch, 512 * (ch + 1))
                    e0 = elu_chunk(tmp[0:64, :], Fst[0][:, sl], 64)
                    nc.vector.tensor_scalar(out=ost[:], in0=e0,
                                            scalar1=float(etw1[0]), scalar2=float(etb1),
                                            op0=OP.mult, op1=OP.add)
                    e1 = elu_chunk(tmp[0:64, :], Fst[1][:, sl], 64)
                    nc.vector.scalar_tensor_tensor(out=ost[:], in0=e1,
                                                   scalar=float(etw1[1]), in1=ost[:],
                                                   op0=OP.mult, op1=OP.add)
                    elu_chunk(ost[:], ost[:], 64)
                    nc.sync.dma_start(outD.ap()[:, sl], ost[:])

    return nc


# ----------------------------------------------------------------------------
# host entry
# ----------------------------------------------------------------------------
def kernel(X, edges0, edges1, W0, a0, W1, a1, etw0, etb0, etw1, etb1):
    X = np.asarray(X, np.float32)
    W0 = np.asarray(W0, np.float32); a0 = np.asarray(a0, np.float32)
    W1 = np.asarray(W1, np.float32); a1 = np.asarray(a1, np.float32)
    etw0 = np.asarray(etw0, np.float32); etb0 = np.asarray(etb0, np.float32)
    etw1 = np.asarray(etw1, np.float32); etb1 = np.asarray(etb1, np.float32)

    offs0 = _derive_offsets(edges0)
    offs1 = _derive_offsets(edges1)
    if offs0 is None or offs1 is None:
        return _np_forward(X, np.asarray(edges0), np.asarray(edges1),
                           W0, a0, W1, a1, etw0, etb0, etw1, etb1)

    from concourse import bass_utils

    key = (tuple(offs0), tuple(offs1), float(etw0[0]), float(etw0[1]),
           float(etb0[0]), float(etw1[0]), float(etw1[1]), float(etb1[0]))
    if key not in _CACHE:
        _CACHE[key] = _build_bass(offs0, offs1,
                                  etw0, float(etb0[0]), etw1, float(etb1[0]))
    nc = _CACHE[key]

    # ---- host-side parameter folding ----
    ins_common = {}
    for j in range(2):
        for p in range(2):
            ins_common[f'w0_{j}_{p}'] = np.ascontiguousarray(
                np.concatenate([W0[j, 2 * p], W0[j, 2 * p + 1]], axis=1))
            ins_common[f'w1_{j}_{p}'] = np.ascontiguousarray(
                np.concatenate([W1[j, 2 * p], W1[j, 2 * p + 1]], axis=1))
    wa0 = np.zeros((IN_DIM, 16), np.float32)
    wa1 = np.zeros((HID, 16), np.float32)
    for j in range(2):
        for h in range(HEADS):
            c = (4 * j + h) * 2
            wa0[:, c] = W0[j, h] @ a0[j, h, :HD]
            wa0[:, c + 1] = W0[j, h] @ a0[j, h, HD:]
            wa1[:, c] = W1[j, h] @ a1[j, h, :OUT_DIM]
            wa1[:, c + 1] = W1[j, h] @ a1[j, h, OUT_DIM:]
    ins_common['wa0'] = wa0
    ins_common['wa1'] = wa1
    selK = np.zeros((64, DEG * 128), np.float32)
    for k in range(DEG):
        selK[k, 128 * k:128 * k + 64] = 1.0
        selK[32 + k, 128 * k + 64:128 * k + 128] = 1.0
    ins_common['selK'] = selK
    fold16 = np.zeros((64, 2), np.float32)
    fold16[0:16, 0] = 1.0
    fold16[32:48, 1] = 1.0
    ins_common['fold16'] = fold16
    fold128 = np.zeros((128, 64), np.float32)
    fold128[np.arange(64), np.arange(64)] = 0.25
    fold128[64 + np.arange(64), np.arange(64)] = 0.25
    ins_common['fold128'] = fold128

    in_maps = []
    for c in range(NCORES):
        r0 = c * ROWS
        idx = (r0 + np.arange(L0)) % N
        m = dict(ins_common)
        m['Xe'] = np.ascontiguousarray(X[idx])
        in_maps.append(m)

    res = bass_utils.run_bass_kernel_spmd(nc, in_maps, core_ids=list(range(NCORES)))
    out = np.empty((N, OUT_DIM), np.float32)
    for c in range(NCORES):
        out[c * ROWS:(c + 1) * ROWS] = res.results[c]['outT'][:, :ROWS].T
    return out
